# revision 1
# baseline (speedup 1.0000x reference)
"""CBOW negative-sampling loss kernel for Trainium2 (8 NeuronCores).

Problem: nn_CBOWModel_18356690223611
    pos_u  [16384, 10] int  -- context word ids into u_weight
    pos_w  [16384]     int  -- target word ids into w_weight
    neg_w  [16384, 5]  int  -- negative sample ids into w_weight
    u_weight [100000, 128] f32
    w_weight [100000, 128] f32
    out = sum_b softplus(-dot(su_b, wpos_b)) + softplus(dot(su_b, wneg_sum_b))
      where su_b = sum_c u_weight[pos_u[b,c]], wneg_sum_b = sum_k w_weight[neg_w[b,k]]
    (equivalent to -(sum logsigmoid(pos) + sum logsigmoid(-neg)))

Sharding: data-parallel over batch, 2048 samples per core; embedding tables
replicated to each core's DRAM. Gathers via SWDGE indirect DMA (one 512B row
descriptor per embedding lookup), reductions/dots on DVE, softplus tail on
ACT, cross-partition sum on PE.
"""

import numpy as np

VOCAB = 100000
DIM = 128
B = 16384
CTX = 10
NEG = 5
WK = NEG + 1  # pos + neg lookups into w_weight per sample

N_CORES = 8
BPC = B // N_CORES  # 2048 samples per core
P = 128
TILES = BPC // P  # 16 sample columns of 128 samples
CHUNK_T = 4  # sample columns per pipeline chunk
N_CHUNKS = TILES // CHUNK_T  # 4

_CACHE = {}


def _build_nc():
    import concourse.bacc as bacc
    import concourse.bass as bass
    import concourse.mybir as mybir
    import concourse.tile as tile

    f32 = mybir.dt.float32
    i32 = mybir.dt.int32
    ADD = mybir.AluOpType.add
    MUL = mybir.AluOpType.mult

    nc = bacc.Bacc("TRN2", target_bir_lowering=False, debug=False,
                   enable_asserts=False)

    idx_d = nc.dram_tensor("idx", [P, (CTX + WK) * TILES], i32,
                           kind="ExternalInput")
    u_w = nc.dram_tensor("u_weight", [VOCAB, DIM], f32, kind="ExternalInput")
    w_w = nc.dram_tensor("w_weight", [VOCAB, DIM], f32, kind="ExternalInput")
    out_d = nc.dram_tensor("out", [1, 1], f32, kind="ExternalOutput")

    with tile.TileContext(nc) as tc:
        with (
            tc.tile_pool(name="idx", bufs=1) as idxp,
            tc.tile_pool(name="gu", bufs=2) as upool,
            tc.tile_pool(name="gw", bufs=2) as wpool,
            tc.tile_pool(name="work", bufs=2) as work,
            tc.tile_pool(name="accum", bufs=1) as accp,
            tc.tile_pool(name="psum", bufs=1, space="PSUM") as psp,
        ):
            idx_t = idxp.tile([P, (CTX + WK) * TILES], i32)
            # single idx load on the gpsimd (Pool) queue: Pool's preamble ends
            # earliest, and one DMA means one descgen before gathers start
            nc.gpsimd.dma_start(out=idx_t[:], in_=idx_d.ap())
            idx_u = idx_t[:, 0:CTX * TILES]
            idx_w = idx_t[:, CTX * TILES:(CTX + WK) * TILES]

            # scores[p, m, k, t]: k=0 -> -pos_score, k=1 -> +neg_score
            scores = accp.tile([P, N_CHUNKS * 2 * CHUNK_T], f32)
            scores_v = scores[:].rearrange("p (m k t) -> p m k t", m=N_CHUNKS, k=2)

            for m in range(N_CHUNKS):
                t0 = m * CHUNK_T
                t1 = t0 + CHUNK_T

                u_t = upool.tile([P, CTX * CHUNK_T * DIM], f32, tag="u_t")
                w_t = wpool.tile([P, WK * CHUNK_T * DIM], f32, tag="w_t")
                u4 = u_t[:].rearrange("p (c t d) -> p c t d", c=CTX, t=CHUNK_T)
                w4 = w_t[:].rearrange("p (c t d) -> p c t d", c=WK, t=CHUNK_T)
                # one indirect DMA per (c, t): the SWDGE consumes exactly one
                # index per dest partition, so each gather is 128 rows
                for c in range(CTX):
                    for t in range(CHUNK_T):
                        blk = (c * CHUNK_T + t) * DIM
                        nc.gpsimd.indirect_dma_start(
                            out=u_t[:, blk:blk + DIM],
                            out_offset=None,
                            in_=u_w.ap(),
                            in_offset=bass.IndirectOffsetOnAxis(
                                ap=idx_u[:, c * TILES + t0 + t:c * TILES + t0 + t + 1],
                                axis=0),
                        )
                for c in range(WK):
                    for t in range(CHUNK_T):
                        blk = (c * CHUNK_T + t) * DIM
                        nc.gpsimd.indirect_dma_start(
                            out=w_t[:, blk:blk + DIM],
                            out_offset=None,
                            in_=w_w.ap(),
                            in_offset=bass.IndirectOffsetOnAxis(
                                ap=idx_w[:, c * TILES + t0 + t:c * TILES + t0 + t + 1],
                                axis=0),
                        )

                # context sum over c=10: tree 10 -> 5 -> (4->2->1) + leftover
                s1 = work.tile([P, 5 * CHUNK_T * DIM], f32, tag="s1")
                s1v = s1[:].rearrange("p (c t d) -> p c t d", c=5, t=CHUNK_T)
                nc.vector.tensor_tensor(out=s1v[:, :, :, :], in0=u4[:, 0:5], in1=u4[:, 5:10], op=ADD)
                s2 = work.tile([P, 2 * CHUNK_T * DIM], f32, tag="s2")
                s2v = s2[:].rearrange("p (c t d) -> p c t d", c=2, t=CHUNK_T)
                nc.vector.tensor_tensor(out=s2v[:, :, :, :], in0=s1v[:, 0:2], in1=s1v[:, 2:4], op=ADD)
                s3 = work.tile([P, CHUNK_T * DIM], f32, tag="s3")
                s3v = s3[:].rearrange("p (o t d) -> p o t d", o=1, t=CHUNK_T)
                nc.vector.tensor_tensor(out=s3v[:, :, :, :], in0=s2v[:, 0:1], in1=s2v[:, 1:2], op=ADD)
                su = work.tile([P, CHUNK_T * DIM], f32, tag="su")
                suv = su[:].rearrange("p (o t d) -> p o t d", o=1, t=CHUNK_T)
                nc.vector.tensor_tensor(out=suv[:, :, :, :], in0=s3v[:, :, :, :], in1=s1v[:, 4:5], op=ADD)

                # negative-sample sum over c=1..5: 4 -> 2 -> 1, + leftover
                n1 = work.tile([P, 2 * CHUNK_T * DIM], f32, tag="n1")
                n1v = n1[:].rearrange("p (c t d) -> p c t d", c=2, t=CHUNK_T)
                nc.vector.tensor_tensor(out=n1v[:, :, :, :], in0=w4[:, 1:3], in1=w4[:, 3:5], op=ADD)
                n2 = work.tile([P, CHUNK_T * DIM], f32, tag="n2")
                n2v = n2[:].rearrange("p (o t d) -> p o t d", o=1, t=CHUNK_T)
                nc.vector.tensor_tensor(out=n2v[:, :, :, :], in0=n1v[:, 0:1], in1=n1v[:, 1:2], op=ADD)
                wneg = work.tile([P, CHUNK_T * DIM], f32, tag="wneg")
                wnv = wneg[:].rearrange("p (o t d) -> p o t d", o=1, t=CHUNK_T)
                nc.vector.tensor_tensor(out=wnv[:, :, :, :], in0=n2v[:, :, :, :], in1=w4[:, 5:6], op=ADD)

                # per-sample dot products
                prod = work.tile([P, 2 * CHUNK_T * DIM], f32, tag="prod")
                pv = prod[:].rearrange("p (k t d) -> p k t d", k=2, t=CHUNK_T)
                nc.vector.tensor_tensor(out=pv[:, 0:1], in0=suv[:, :, :, :], in1=w4[:, 0:1], op=MUL)
                nc.vector.tensor_tensor(out=pv[:, 1:2], in0=suv[:, :, :, :], in1=wnv[:, :, :, :], op=MUL)
                nc.vector.tensor_reduce(
                    out=scores_v[:, m:m + 1, 0:1, :], in_=pv[:, 0:1],
                    axis=mybir.AxisListType.X, op=ADD, negate=True)
                nc.vector.tensor_reduce(
                    out=scores_v[:, m:m + 1, 1:2, :], in_=pv[:, 1:2],
                    axis=mybir.AxisListType.X, op=ADD)

            # tail: res = sum_{p,i} softplus(scores[p,i]), overflow-safe:
            # softplus(x) = relu(x) + log1p(exp(-|x|))
            NS = N_CHUNKS * 2 * CHUNK_T
            relu = accp.tile([P, NS], f32)
            nc.vector.tensor_scalar_max(relu[:], scores[:], 0.0)
            tmp = accp.tile([P, NS], f32)
            nc.vector.tensor_tensor(out=tmp[:], in0=scores[:], in1=relu[:],
                                    op=mybir.AluOpType.subtract)  # min(x, 0)
            nabs = accp.tile([P, NS], f32)
            nc.vector.tensor_tensor(out=nabs[:], in0=tmp[:], in1=relu[:],
                                    op=mybir.AluOpType.subtract)  # -|x|
            ex = accp.tile([P, NS], f32)
            nc.scalar.activation(ex[:], nabs[:], mybir.ActivationFunctionType.Exp)
            ln = accp.tile([P, NS], f32)
            nc.scalar.activation(ln[:], ex[:], mybir.ActivationFunctionType.Ln,
                                 bias=1.0)
            sp = accp.tile([P, NS], f32)
            nc.vector.tensor_tensor(out=sp[:], in0=relu[:], in1=ln[:], op=ADD)
            row = accp.tile([P, 1], f32)
            nc.vector.tensor_reduce(out=row[:], in_=sp[:],
                                    axis=mybir.AxisListType.X, op=ADD)

            # cross-partition sum: [1,1] = row.T @ ones
            ones = accp.tile([P, 1], f32)
            nc.vector.memset(ones[:], 1.0)
            ps = psp.tile([1, 1], f32)
            nc.tensor.matmul(ps[:], lhsT=row[:], rhs=ones[:], start=True, stop=True)
            res_sb = accp.tile([1, 1], f32)
            nc.vector.tensor_copy(out=res_sb[:], in_=ps[:])
            nc.sync.dma_start(out=out_d.ap(), in_=res_sb[:])

    # Exp and Ln both live in the natural_log_exp_and_others table set, but
    # the greedy table chooser picks exp_and_others for Exp and natural_log
    # for Ln, putting a ~2.7us table swap in the kernel's serial tail. Empty
    # those two sets (positions preserved -- act_func_set_id is positional)
    # during compile so both funcs resolve to the combined table.
    orig_tables = bacc.get_activation_tables

    def _tables_combined(arch):
        t = dict(orig_tables(arch))
        if "natural_log_exp_and_others" in t:
            for k in ("exp_and_others", "natural_log"):
                if k in t:
                    t[k] = frozenset()
        return t

    bacc.get_activation_tables = _tables_combined
    try:
        nc.compile()
    finally:
        bacc.get_activation_tables = orig_tables
    return nc


def _get_nc():
    if "nc" not in _CACHE:
        _CACHE["nc"] = _build_nc()
    return _CACHE["nc"]


def _make_in_maps(pos_u, pos_w, neg_w, u_weight, w_weight):
    pos_u = np.asarray(pos_u)
    pos_w = np.asarray(pos_w)
    neg_w = np.asarray(neg_w)
    u_weight = np.ascontiguousarray(np.asarray(u_weight, dtype=np.float32))
    w_weight = np.ascontiguousarray(np.asarray(w_weight, dtype=np.float32))

    in_maps = []
    for c in range(N_CORES):
        sl = slice(c * BPC, (c + 1) * BPC)
        pu = np.asarray(pos_u[sl], dtype=np.int32)  # [2048, 10]
        # device layout [p, c, t]: sample s = t*128 + p
        iu = pu.reshape(TILES, P, CTX).transpose(1, 2, 0)
        wind = np.concatenate(
            [np.asarray(pos_w[sl], dtype=np.int32)[:, None],
             np.asarray(neg_w[sl], dtype=np.int32)], axis=1)  # [2048, 6]
        iw = wind.reshape(TILES, P, WK).transpose(1, 2, 0)
        idx_all = np.concatenate(
            [np.ascontiguousarray(iu).reshape(P, CTX * TILES),
             np.ascontiguousarray(iw).reshape(P, WK * TILES)], axis=1)
        in_maps.append({
            "idx": np.ascontiguousarray(idx_all),
            "u_weight": u_weight,
            "w_weight": w_weight,
        })
    return in_maps


def kernel(pos_u, pos_w, neg_w, u_weight, w_weight):
    from concourse.bass_utils import run_bass_kernel_spmd

    nc = _get_nc()
    in_maps = _make_in_maps(pos_u, pos_w, neg_w, u_weight, w_weight)
    res = run_bass_kernel_spmd(nc, in_maps, core_ids=list(range(N_CORES)))
    total = sum(float(r["out"][0, 0]) for r in res.results)
    return np.asarray(total, dtype=np.float32)



# revision 2
# speedup vs baseline: 5.0786x; 5.0786x over previous
"""CBOW negative-sampling loss kernel for Trainium2 (8 NeuronCores).

Problem: nn_CBOWModel_18356690223611
    pos_u  [16384, 10] int  -- context word ids into u_weight
    pos_w  [16384]     int  -- target word ids into w_weight
    neg_w  [16384, 5]  int  -- negative sample ids into w_weight
    u_weight [100000, 128] f32
    w_weight [100000, 128] f32
    out = sum_b softplus(-dot(su_b, wpos_b)) + softplus(dot(su_b, wneg_sum_b))
      where su_b = sum_c u_weight[pos_u[b,c]], wneg_sum_b = sum_k w_weight[neg_w[b,k]]
    (equivalent to -(sum logsigmoid(pos) + sum logsigmoid(-neg)))

Sharding: data-parallel over batch, 2048 samples per core; embedding tables
replicated (concatenated into one [200000, 128] DRAM tensor) per core.

Gathers: ONE SWDGE indirect DMA per 512-sample chunk (8192 rows of 512B per
instruction) instead of one per 128 rows -- the ~1us fixed SWDGE descriptor
generation cost per instruction was the baseline bottleneck (GpSimd 295us
busy vs 16 DMA engines each <20% busy). The indirect DMA pairs gathered rows
with indices in AP-walk order: dest block j of partition p <- idx[p, j].
Reductions/dots on DVE, softplus tail on ACT, cross-partition sum on PE.
"""

import numpy as np

VOCAB = 100000
DIM = 128
B = 16384
CTX = 10
NEG = 5
WK = NEG + 1  # pos + neg lookups into w_weight per sample
NIDX = CTX + WK  # 16 gathered rows per sample

N_CORES = 8
BPC = B // N_CORES  # 2048 samples per core
P = 128
TILES = BPC // P  # 16 sample columns of 128 samples
CHUNK_T = 4  # sample columns per pipeline chunk
N_CHUNKS = TILES // CHUNK_T  # 4

_CACHE = {}


def _build_nc():
    import concourse.bacc as bacc
    import concourse.bass as bass
    import concourse.mybir as mybir
    import concourse.tile as tile

    f32 = mybir.dt.float32
    i32 = mybir.dt.int32
    ADD = mybir.AluOpType.add
    MUL = mybir.AluOpType.mult

    nc = bacc.Bacc("TRN2", target_bir_lowering=False, debug=False,
                   enable_asserts=False)

    idx_d = nc.dram_tensor("idx", [P, NIDX * TILES], i32,
                           kind="ExternalInput")
    uw_w = nc.dram_tensor("uw_weight", [2 * VOCAB, DIM], f32,
                          kind="ExternalInput")
    out_d = nc.dram_tensor("out", [1, 1], f32, kind="ExternalOutput")

    CHUNK_COLS = NIDX * CHUNK_T  # 64 gathered rows per partition per chunk

    with tile.TileContext(nc) as tc:
        with (
            tc.tile_pool(name="idx", bufs=1) as idxp,
            tc.tile_pool(name="g", bufs=2) as gpool,
            tc.tile_pool(name="work", bufs=2) as work,
            tc.tile_pool(name="accum", bufs=1) as accp,
            tc.tile_pool(name="psum", bufs=1, space="PSUM") as psp,
        ):
            idx_t = idxp.tile([P, NIDX * TILES], i32)
            # single idx load on the gpsimd (Pool) queue: Pool's preamble ends
            # earliest, and one DMA means one descgen before gathers start
            nc.gpsimd.dma_start(out=idx_t[:], in_=idx_d.ap())

            # scores[p, m, k, t]: k=0 -> -pos_score, k=1 -> +neg_score
            scores = accp.tile([P, N_CHUNKS * 2 * CHUNK_T], f32)
            scores_v = scores[:].rearrange("p (m k t) -> p m k t", m=N_CHUNKS, k=2)

            for m in range(N_CHUNKS):
                # one gather for the whole chunk: 64 rows per partition, in
                # host-prepared order (c-major, t-minor; u rows then w rows)
                g_t = gpool.tile([P, CHUNK_COLS * DIM], f32, tag="g_t")
                nc.gpsimd.indirect_dma_start(
                    out=g_t[:],
                    out_offset=None,
                    in_=uw_w.ap(),
                    in_offset=bass.IndirectOffsetOnAxis(
                        ap=idx_t[:, m * CHUNK_COLS:(m + 1) * CHUNK_COLS],
                        axis=0),
                )
                u4 = g_t[:, 0:CTX * CHUNK_T * DIM].rearrange(
                    "p (c t d) -> p c t d", c=CTX, t=CHUNK_T)
                w4 = g_t[:, CTX * CHUNK_T * DIM:CHUNK_COLS * DIM].rearrange(
                    "p (c t d) -> p c t d", c=WK, t=CHUNK_T)

                # context sum over c=10: tree 10 -> 5 -> (4->2->1) + leftover
                s1 = work.tile([P, 5 * CHUNK_T * DIM], f32, tag="s1")
                s1v = s1[:].rearrange("p (c t d) -> p c t d", c=5, t=CHUNK_T)
                nc.vector.tensor_tensor(out=s1v[:, :, :, :], in0=u4[:, 0:5], in1=u4[:, 5:10], op=ADD)
                s2 = work.tile([P, 2 * CHUNK_T * DIM], f32, tag="s2")
                s2v = s2[:].rearrange("p (c t d) -> p c t d", c=2, t=CHUNK_T)
                nc.vector.tensor_tensor(out=s2v[:, :, :, :], in0=s1v[:, 0:2], in1=s1v[:, 2:4], op=ADD)
                s3 = work.tile([P, CHUNK_T * DIM], f32, tag="s3")
                s3v = s3[:].rearrange("p (o t d) -> p o t d", o=1, t=CHUNK_T)
                nc.vector.tensor_tensor(out=s3v[:, :, :, :], in0=s2v[:, 0:1], in1=s2v[:, 1:2], op=ADD)
                su = work.tile([P, CHUNK_T * DIM], f32, tag="su")
                suv = su[:].rearrange("p (o t d) -> p o t d", o=1, t=CHUNK_T)
                nc.vector.tensor_tensor(out=suv[:, :, :, :], in0=s3v[:, :, :, :], in1=s1v[:, 4:5], op=ADD)

                # negative-sample sum over c=1..5: 4 -> 2 -> 1, + leftover
                n1 = work.tile([P, 2 * CHUNK_T * DIM], f32, tag="n1")
                n1v = n1[:].rearrange("p (c t d) -> p c t d", c=2, t=CHUNK_T)
                nc.vector.tensor_tensor(out=n1v[:, :, :, :], in0=w4[:, 1:3], in1=w4[:, 3:5], op=ADD)
                n2 = work.tile([P, CHUNK_T * DIM], f32, tag="n2")
                n2v = n2[:].rearrange("p (o t d) -> p o t d", o=1, t=CHUNK_T)
                nc.vector.tensor_tensor(out=n2v[:, :, :, :], in0=n1v[:, 0:1], in1=n1v[:, 1:2], op=ADD)
                wneg = work.tile([P, CHUNK_T * DIM], f32, tag="wneg")
                wnv = wneg[:].rearrange("p (o t d) -> p o t d", o=1, t=CHUNK_T)
                nc.vector.tensor_tensor(out=wnv[:, :, :, :], in0=n2v[:, :, :, :], in1=w4[:, 5:6], op=ADD)

                # per-sample dot products
                prod = work.tile([P, 2 * CHUNK_T * DIM], f32, tag="prod")
                pv = prod[:].rearrange("p (k t d) -> p k t d", k=2, t=CHUNK_T)
                nc.vector.tensor_tensor(out=pv[:, 0:1], in0=suv[:, :, :, :], in1=w4[:, 0:1], op=MUL)
                nc.vector.tensor_tensor(out=pv[:, 1:2], in0=suv[:, :, :, :], in1=wnv[:, :, :, :], op=MUL)
                nc.vector.tensor_reduce(
                    out=scores_v[:, m:m + 1, 0:1, :], in_=pv[:, 0:1],
                    axis=mybir.AxisListType.X, op=ADD, negate=True)
                nc.vector.tensor_reduce(
                    out=scores_v[:, m:m + 1, 1:2, :], in_=pv[:, 1:2],
                    axis=mybir.AxisListType.X, op=ADD)

            # tail: res = sum_{p,i} softplus(scores[p,i]), overflow-safe:
            # softplus(x) = relu(x) + log1p(exp(-|x|))
            NS = N_CHUNKS * 2 * CHUNK_T
            relu = accp.tile([P, NS], f32)
            nc.vector.tensor_scalar_max(relu[:], scores[:], 0.0)
            tmp = accp.tile([P, NS], f32)
            nc.vector.tensor_tensor(out=tmp[:], in0=scores[:], in1=relu[:],
                                    op=mybir.AluOpType.subtract)  # min(x, 0)
            nabs = accp.tile([P, NS], f32)
            nc.vector.tensor_tensor(out=nabs[:], in0=tmp[:], in1=relu[:],
                                    op=mybir.AluOpType.subtract)  # -|x|
            ex = accp.tile([P, NS], f32)
            nc.scalar.activation(ex[:], nabs[:], mybir.ActivationFunctionType.Exp)
            ln = accp.tile([P, NS], f32)
            nc.scalar.activation(ln[:], ex[:], mybir.ActivationFunctionType.Ln,
                                 bias=1.0)
            sp = accp.tile([P, NS], f32)
            nc.vector.tensor_tensor(out=sp[:], in0=relu[:], in1=ln[:], op=ADD)
            row = accp.tile([P, 1], f32)
            nc.vector.tensor_reduce(out=row[:], in_=sp[:],
                                    axis=mybir.AxisListType.X, op=ADD)

            # cross-partition sum: [1,1] = row.T @ ones
            ones = accp.tile([P, 1], f32)
            nc.vector.memset(ones[:], 1.0)
            ps = psp.tile([1, 1], f32)
            nc.tensor.matmul(ps[:], lhsT=row[:], rhs=ones[:], start=True, stop=True)
            res_sb = accp.tile([1, 1], f32)
            nc.vector.tensor_copy(out=res_sb[:], in_=ps[:])
            nc.sync.dma_start(out=out_d.ap(), in_=res_sb[:])

    # Exp and Ln both live in the natural_log_exp_and_others table set, but
    # the greedy table chooser picks exp_and_others for Exp and natural_log
    # for Ln, putting a ~2.7us table swap in the kernel's serial tail. Empty
    # those two sets (positions preserved -- act_func_set_id is positional)
    # during compile so both funcs resolve to the combined table.
    orig_tables = bacc.get_activation_tables

    def _tables_combined(arch):
        t = dict(orig_tables(arch))
        if "natural_log_exp_and_others" in t:
            for k in ("exp_and_others", "natural_log"):
                if k in t:
                    t[k] = frozenset()
        return t

    bacc.get_activation_tables = _tables_combined
    try:
        nc.compile()
    finally:
        bacc.get_activation_tables = orig_tables
    return nc


def _get_nc():
    if "nc" not in _CACHE:
        _CACHE["nc"] = _build_nc()
    return _CACHE["nc"]


def _make_in_maps(pos_u, pos_w, neg_w, u_weight, w_weight):
    pos_u = np.asarray(pos_u)
    pos_w = np.asarray(pos_w)
    neg_w = np.asarray(neg_w)
    uw = np.ascontiguousarray(
        np.concatenate([np.asarray(u_weight, dtype=np.float32),
                        np.asarray(w_weight, dtype=np.float32)], axis=0))

    in_maps = []
    for c in range(N_CORES):
        sl = slice(c * BPC, (c + 1) * BPC)
        # per-sample 16 indices: u c=0..9 then w k=0..5 (+VOCAB offset into
        # the concatenated table)
        all_ind = np.concatenate(
            [np.asarray(pos_u[sl], dtype=np.int32),
             np.asarray(pos_w[sl], dtype=np.int32)[:, None] + VOCAB,
             np.asarray(neg_w[sl], dtype=np.int32) + VOCAB], axis=1)  # [2048, 16]
        # device layout idx[p, m*64 + j*CHUNK_T + t] for sample
        # s = (m*CHUNK_T + t)*128 + p:  [m, t, p, j] -> [p, m, j, t]
        idx = (all_ind.reshape(N_CHUNKS, CHUNK_T, P, NIDX)
               .transpose(2, 0, 3, 1)
               .reshape(P, N_CHUNKS * NIDX * CHUNK_T))
        in_maps.append({
            "idx": np.ascontiguousarray(idx),
            "uw_weight": uw,
        })
    return in_maps


def kernel(pos_u, pos_w, neg_w, u_weight, w_weight):
    from concourse.bass_utils import run_bass_kernel_spmd

    nc = _get_nc()
    in_maps = _make_in_maps(pos_u, pos_w, neg_w, u_weight, w_weight)
    res = run_bass_kernel_spmd(nc, in_maps, core_ids=list(range(N_CORES)))
    total = sum(float(r["out"][0, 0]) for r in res.results)
    return np.asarray(total, dtype=np.float32)
